# revision 14
# baseline (speedup 1.0000x reference)
"""Trainium2 Bass kernel for nn_AttentionBlock (sliding-window GQA attention block).

Full inputs in, full output out. Internally: tensor-parallel over the 8 KV-head
groups (1 per NeuronCore), partial out-projections summed on host.

Per-core device program (token-major scheme):
  x [2048, 2944pad] --DMA--> SBUF; ACT sumsq -> r = rsqrt(mean+eps)
  PE-transpose x tiles -> xT (f32r);  qkv = xT.T @ W (f32r, norm_scale folded)
  ACT copy psum->SBUF with scale=r (applies rmsnorm)
  DVE rope (tables host-precomputed, SM_SCALE folded into q tables)
  PE-transpose roped q,k -> qT/kT (f32r, d-major)
  scores^T = kT.T @ qT per (q-pair, head): [keys<=3x128, 256] (f32r)
  DVE +mask, ACT exp, PE AV with ones-augmented v -> [attn^T; denom]
  DVE divide by DMA-broadcast denom -> attn^T (bf16)
  out-proj: psum += attnT_k.T @ w_out_k (bf16), DVE copy, DMA partial out.

Host: out = x + sum(partials) + out_b.
"""
import math
import numpy as np

N_TOKENS = 2048
HIDDEN = 2880
HID_PAD = 2944  # 23 * 128
HEAD_DIM = 64
N_HEADS = 64
KV_HEADS = 8
Q_MULT = 8
WINDOW = 128
BASE = 150000.0
INIT_CTX = 4096
ROPE_SCALE = 32.0
NTK_ALPHA = 1.0
NTK_BETA = 32.0
SM_SCALE = 1.0 / math.sqrt(HEAD_DIM)
NEG_INF = -1e30

N_CORES = 8
Q_COLS = N_HEADS * HEAD_DIM          # 4096
KV_COLS = KV_HEADS * HEAD_DIM        # 512
GRP = Q_MULT * HEAD_DIM              # 512 q cols per core
W_G_COLS = GRP + 2 * HEAD_DIM        # 640
N_TT = N_TOKENS // 128               # 16 token tiles
N_PAIR = N_TT // 2                   # 8 q-tile pairs
N_KT = HID_PAD // 128                # 23 hidden k-tiles
QKV_CH = 2                           # 2 x 320 feature chunks
OUT_CH = 6                           # 6 x 480 out-proj chunks
OCH = HIDDEN // OUT_CH               # 480

_CACHE = {}


def _rope_tables():
    # mirror reference._rope_cos_sin bit-for-bit (jnp f32 on CPU)
    import jax
    import jax.numpy as jnp
    with jax.default_device(jax.devices("cpu")[0]):
        return _rope_tables_impl(jnp)


def _rope_tables_impl(jnp):
    d_half = HEAD_DIM / 2
    freq = BASE ** (jnp.arange(0, HEAD_DIM, 2, dtype=jnp.float32) / HEAD_DIM)
    concentration = 0.1 * math.log(ROPE_SCALE) + 1.0
    low = d_half * math.log(INIT_CTX / (NTK_BETA * 2 * math.pi)) / math.log(BASE)
    high = d_half * math.log(INIT_CTX / (NTK_ALPHA * 2 * math.pi)) / math.log(BASE)
    interpolation = 1.0 / (ROPE_SCALE * freq)
    extrapolation = 1.0 / freq
    ramp = (jnp.arange(d_half, dtype=jnp.float32) - low) / (high - low)
    mask = 1.0 - jnp.clip(ramp, 0.0, 1.0)
    inv_freq = interpolation * (1.0 - mask) + extrapolation * mask
    t = jnp.arange(N_TOKENS, dtype=jnp.float32)
    freqs = t[:, None] * inv_freq[None, :]
    cos = np.asarray(jnp.cos(freqs) * concentration, dtype=np.float32)
    sin = np.asarray(jnp.sin(freqs) * concentration, dtype=np.float32)
    return cos, sin


def _mask3():
    # maskB[j, i, u]: additive mask for scores^T block layout
    # key tile kt = 2p-1+i, key j in tile; query u in pair (2 tiles)
    j = np.arange(128)[:, None, None]
    i = np.arange(3)[None, :, None]
    u = np.arange(256)[None, None, :]
    d = u - j + (1 - i) * 128  # qi - kj
    allowed = (d >= 0) & (d <= WINDOW - 1)
    return np.where(allowed, 0.0, NEG_INF).astype(np.float32)


def _build_program(loop_n=1, debug=False):
    import concourse.bacc as bacc
    import concourse.mybir as mybir
    from concourse.tile import TileContext

    F32 = mybir.dt.float32
    F32R = mybir.dt.float32r
    BF16 = mybir.dt.bfloat16
    MUL = mybir.AluOpType.mult
    ADD = mybir.AluOpType.add
    SUB = mybir.AluOpType.subtract
    DIV = mybir.AluOpType.divide
    EXP = mybir.ActivationFunctionType.Exp
    SQUARE = mybir.ActivationFunctionType.Square
    SQRT = mybir.ActivationFunctionType.Sqrt

    nc = bacc.Bacc("TRN2", target_bir_lowering=False, debug=False)

    x_d = nc.dram_tensor("x", (N_TOKENS, HID_PAD), F32R, kind="ExternalInput").ap()
    wq_d = nc.dram_tensor("w_qkv", (HID_PAD, W_G_COLS), F32R, kind="ExternalInput").ap()
    wo_d = nc.dram_tensor("w_out", (GRP, HIDDEN), BF16, kind="ExternalInput").ap()
    cq_d = nc.dram_tensor("cos_q", (N_TOKENS, 32), F32, kind="ExternalInput").ap()
    sq_d = nc.dram_tensor("sin_q", (N_TOKENS, 32), F32, kind="ExternalInput").ap()
    ck_d = nc.dram_tensor("cos_k", (N_TOKENS, 32), F32, kind="ExternalInput").ap()
    sk_d = nc.dram_tensor("sin_k", (N_TOKENS, 32), F32, kind="ExternalInput").ap()
    mk_d = nc.dram_tensor("mask3", (128, 3, 256), F32, kind="ExternalInput").ap()
    es_d = nc.dram_tensor("esink", (128, Q_MULT), F32, kind="ExternalInput").ap()
    id_d = nc.dram_tensor("ident", (128, 128), F32R, kind="ExternalInput").ap()
    on_d = nc.dram_tensor("ones", (128, 1), F32R, kind="ExternalInput").ap()
    po_d = nc.dram_tensor("pout", (N_TOKENS, HIDDEN), F32, kind="ExternalOutput").ap()
    dbg = {}
    if debug:
        for nm, shp in (("dbg_qkv", (128, W_G_COLS)), ("dbg_qro", (128, GRP)),
                        ("dbg_kro", (128, HEAD_DIM)), ("dbg_qT", (64, Q_MULT, 256)),
                        ("dbg_kT", (64, 128)), ("dbg_eT", (128, 3, 256)),
                        ("dbg_den", (128, 256)), ("dbg_denbc", (64, 256)),
                        ("dbg_attn", (128, 4, 256)), ("dbg_xT", (128, 128)),
                        ("dbg_r", (128, 1))):
            dbg[nm] = nc.dram_tensor(nm, shp, F32, kind="ExternalOutput").ap()

    with TileContext(nc) as tc:
        with tc.tile_pool(name="const", bufs=1) as cpool, \
             tc.tile_pool(name="work", bufs=2) as wp, \
             tc.tile_pool(name="xtp", bufs=1) as xtp, \
             tc.tile_pool(name="kv", bufs=4) as kvp, \
             tc.tile_pool(name="ps_xp", bufs=1, space="PSUM") as ps_xp, \
             tc.tile_pool(name="ps_qkv", bufs=2, space="PSUM") as ps_qkv, \
             tc.tile_pool(name="ps_sc", bufs=1, space="PSUM") as ps_sc, \
             tc.tile_pool(name="ps_av", bufs=1, space="PSUM") as ps_av, \
             tc.tile_pool(name="ps_op", bufs=2, space="PSUM") as ps_op:

            # ---- resident tiles ----
            wq_sb = cpool.tile([128, N_KT, W_G_COLS], F32R, tag="wq")
            for kt in range(N_KT):
                nc.sync.dma_start(wq_sb[:, kt, :], wq_d[kt * 128:(kt + 1) * 128, :])
            wo_sb = cpool.tile([128, 4, HIDDEN], BF16, tag="wo")
            for kt in range(4):
                nc.sync.dma_start(wo_sb[:, kt, :], wo_d[kt * 128:(kt + 1) * 128, :])
            cq_sb = cpool.tile([128, N_TT, 32], F32, tag="cq")
            sq_sb = cpool.tile([128, N_TT, 32], F32, tag="sq")
            ck_sb = cpool.tile([128, N_TT, 32], F32, tag="ck")
            sk_sb = cpool.tile([128, N_TT, 32], F32, tag="sk")
            for sb_t, dr in ((cq_sb, cq_d), (sq_sb, sq_d), (ck_sb, ck_d), (sk_sb, sk_d)):
                nc.sync.dma_start(sb_t[:], dr.rearrange("(t p) d -> p t d", p=128))
            mk_sb = cpool.tile([128, 3, 256], F32, tag="mk")
            nc.sync.dma_start(mk_sb[:], mk_d)
            es_sb = cpool.tile([128, Q_MULT], F32, tag="es")
            nc.sync.dma_start(es_sb[:], es_d)
            id_sb = cpool.tile([128, 128], F32R, tag="id")
            nc.sync.dma_start(id_sb[:], id_d)
            eps_sb = cpool.tile([128, 1], F32, tag="eps")
            nc.vector.memset(eps_sb[:], 1e-5)
            ones_sb = cpool.tile([128, 1], F32R, tag="ones")
            nc.sync.dma_start(ones_sb[:], on_d)

            kT_tiles = [None] * N_TT
            vA_tiles = [None] * N_TT
            qT_pairs = [None] * N_PAIR
            attn_pairs = [None] * N_PAIR

            def produce_tile(tt):
                """DMA x tile, rmsnorm stats, transpose, qkv, rope, re-transpose."""
                x_sb = wp.tile([128, HID_PAD], F32R, tag="x")
                nc.sync.dma_start(x_sb[:], x_d[tt * 128:(tt + 1) * 128, :])

                # rmsnorm scale r = 1/sqrt(mean(x^2)+eps) ; 4 chunks into scratch
                sumsq = wp.tile([128, 4], F32, tag="sumsq")
                scr = xtp.tile([128, 736], F32, tag="xsq_scratch")
                for ch in range(4):
                    nc.scalar.activation(
                        scr[:], x_sb[:, ch * 736:(ch + 1) * 736].bitcast(F32),
                        SQUARE, accum_out=sumsq[:, ch:ch + 1])
                s01 = wp.tile([128, 2], F32, tag="s01")
                nc.vector.tensor_tensor(out=s01[:, 0:1], in0=sumsq[:, 0:1],
                                        in1=sumsq[:, 1:2], op=ADD)
                nc.vector.tensor_tensor(out=s01[:, 1:2], in0=sumsq[:, 2:3],
                                        in1=sumsq[:, 3:4], op=ADD)
                std = wp.tile([128, 1], F32, tag="std")
                nc.vector.tensor_tensor(out=std[:], in0=s01[:, 0:1],
                                        in1=s01[:, 1:2], op=ADD)
                nc.scalar.activation(std[:], std[:], SQRT,
                                     bias=eps_sb[:], scale=1.0 / HIDDEN)
                r_t = wp.tile([128, 1], F32, tag="r")
                nc.vector.reciprocal(r_t[:], std[:])
                if debug and tt == 0:
                    nc.sync.dma_start(dbg["dbg_r"], r_t[:])

                # transpose x -> xT (f32r)
                xT = xtp.tile([128, N_KT, 128], F32R, tag="xT")
                for kt in range(N_KT):
                    xps = ps_xp.tile([128, 128], F32R, tag="xps")
                    nc.tensor.transpose(xps[:], x_sb[:, kt * 128:(kt + 1) * 128],
                                        id_sb[:])
                    nc.scalar.copy(xT[:, kt, :], xps[:])
                    if debug and tt == 0 and kt == 0:
                        nc.sync.dma_start(dbg["dbg_xT"], xT[:, 0, :].bitcast(F32))

                # qkv = xT.T @ W, scaled by r on copy-out
                qkv_sb = wp.tile([128, W_G_COLS], F32, tag="qkv")
                for ch in range(QKV_CH):
                    qps = ps_qkv.tile([128, 320], F32, tag="qps")
                    for kt in range(N_KT):
                        nc.tensor.matmul(qps[:], xT[:, kt, :],
                                         wq_sb[:, kt, ch * 320:(ch + 1) * 320],
                                         start=(kt == 0), stop=(kt == N_KT - 1))
                    nc.scalar.mul(qkv_sb[:, ch * 320:(ch + 1) * 320], qps[:],
                                  mul=r_t[:])

                if debug and tt == 0:
                    nc.sync.dma_start(dbg["dbg_qkv"], qkv_sb[:])
                # rope: q (8 heads), k (1 head)
                q_ro = wp.tile([128, GRP], F32R, tag="q_ro")
                k_ro = wp.tile([128, HEAD_DIM], F32R, tag="k_ro")
                ta = wp.tile([128, Q_MULT, 32], F32, tag="rope_a")
                tb = wp.tile([128, Q_MULT, 32], F32, tag="rope_b")
                q3 = qkv_sb[:, 0:GRP].rearrange("p (h d) -> p h d", h=Q_MULT)
                qo3 = q_ro[:].rearrange("p (h d) -> p h d", h=Q_MULT)
                cqb = cq_sb[:, tt:tt + 1, :].broadcast_to((128, Q_MULT, 32))
                sqb = sq_sb[:, tt:tt + 1, :].broadcast_to((128, Q_MULT, 32))
                nc.vector.tensor_tensor(out=ta[:], in0=q3[:, :, 0:32], in1=cqb, op=MUL)
                nc.vector.tensor_tensor(out=tb[:], in0=q3[:, :, 32:64], in1=sqb, op=MUL)
                nc.vector.tensor_tensor(out=qo3[:, :, 0:32], in0=ta[:], in1=tb[:], op=SUB)
                nc.vector.tensor_tensor(out=ta[:], in0=q3[:, :, 32:64], in1=cqb, op=MUL)
                nc.vector.tensor_tensor(out=tb[:], in0=q3[:, :, 0:32], in1=sqb, op=MUL)
                nc.vector.tensor_tensor(out=qo3[:, :, 32:64], in0=ta[:], in1=tb[:], op=ADD)
                k2 = qkv_sb[:, GRP:GRP + HEAD_DIM]
                nc.vector.tensor_tensor(out=ta[:, 0, :], in0=k2[:, 0:32],
                                        in1=ck_sb[:, tt, :], op=MUL)
                nc.vector.tensor_tensor(out=tb[:, 0, :], in0=k2[:, 32:64],
                                        in1=sk_sb[:, tt, :], op=MUL)
                nc.vector.tensor_tensor(out=k_ro[:, 0:32], in0=ta[:, 0, :],
                                        in1=tb[:, 0, :], op=SUB)
                nc.vector.tensor_tensor(out=ta[:, 0, :], in0=k2[:, 32:64],
                                        in1=ck_sb[:, tt, :], op=MUL)
                nc.vector.tensor_tensor(out=tb[:, 0, :], in0=k2[:, 0:32],
                                        in1=sk_sb[:, tt, :], op=MUL)
                nc.vector.tensor_tensor(out=k_ro[:, 32:64], in0=ta[:, 0, :],
                                        in1=tb[:, 0, :], op=ADD)

                if debug and tt == 0:
                    nc.sync.dma_start(dbg["dbg_qro"], q_ro[:].bitcast(F32))
                    nc.sync.dma_start(dbg["dbg_kro"], k_ro[:].bitcast(F32))
                # v augmented with ones column (AV denominator trick)
                vA = kvp.tile([128, HEAD_DIM + 1], F32R, tag="vaug")
                nc.vector.tensor_copy(vA[:, 0:HEAD_DIM],
                                      qkv_sb[:, GRP + HEAD_DIM:GRP + 2 * HEAD_DIM])
                nc.vector.tensor_copy(vA[:, HEAD_DIM:HEAD_DIM + 1], ones_sb[:])
                vA_tiles[tt] = vA

                # re-transpose roped q, k to d-major
                p = tt // 2
                if qT_pairs[p] is None:
                    qT_pairs[p] = wp.tile([64, Q_MULT, 256], F32R, tag="qT_pair",
                                          name="qT_pair")
                qT = qT_pairs[p]
                half = (tt % 2) * 128
                for j in range(Q_MULT):
                    tps = ps_xp.tile([128, 128], F32R, tag="xps")
                    nc.tensor.transpose(tps[0:64, :], q_ro[:, j * 64:(j + 1) * 64],
                                        id_sb[:])
                    nc.scalar.copy(qT[:, j, half:half + 128], tps[0:64, :])
                kT = kvp.tile([64, 128], F32R, tag="kT")
                kps = ps_xp.tile([128, 128], F32R, tag="xps")
                nc.tensor.transpose(kps[0:64, :], k_ro[:], id_sb[:])
                nc.scalar.copy(kT[:], kps[0:64, :])
                kT_tiles[tt] = kT
                if debug and tt == 0:
                    nc.sync.dma_start(dbg["dbg_kT"], kT[:].bitcast(F32))
                if debug and tt == 1:
                    nc.sync.dma_start(dbg["dbg_qT"], qT[:].bitcast(F32))

            def attention_pair(p):
                """scores/softmax/AV + normalize for q-tiles 2p, 2p+1."""
                kts = [2 * p - 1 + i for i in range(3)]
                kts = [(i, kt) for i, kt in enumerate(kts) if kt >= 0]
                qT = qT_pairs[p]
                attn = wp.tile([128, 4, 256], BF16, tag="attn_pair")
                attn_pairs[p] = attn
                for h in range(Q_MULT):
                    sps = ps_sc.tile([128, 3, 256], F32, tag="sps")
                    for i, kt in kts:
                        nc.tensor.matmul(sps[:, i, :], kT_tiles[kt][:],
                                         qT[:, h, :],
                                         start=True, stop=True)
                    masked = wp.tile([128, 3, 256], F32, tag="masked")
                    eT = wp.tile([128, 3, 256], F32R, tag="eT")
                    i0 = kts[0][0]
                    nc.vector.tensor_tensor(out=masked[:, i0:3, :],
                                            in0=sps[:, i0:3, :],
                                            in1=mk_sb[:, i0:3, :], op=ADD)
                    nc.scalar.activation(eT[:, i0:3, :], masked[:, i0:3, :], EXP)
                    if debug and p == 0 and h == 0:
                        nc.sync.dma_start(dbg["dbg_eT"], eT[:].bitcast(F32))
                    aps = ps_av.tile([65, 256], F32, tag="aps")
                    for i, kt in kts:
                        nc.tensor.matmul(aps[:], vA_tiles[kt][:], eT[:, i, :],
                                         start=(i == i0), stop=(i == 2))
                    # denom = row 64 + exp(sink); broadcast; divide
                    den = wp.tile([128, 256], F32, tag="den")
                    nc.scalar.activation(den[64:65, :], aps[64:65, :],
                                         mybir.ActivationFunctionType.Identity,
                                         bias=es_sb[64:65, h:h + 1])
                    nc.vector.reciprocal(den[64:65, :], den[64:65, :])
                    den0 = wp.tile([1, 256], F32, tag="den0")
                    nc.sync.dma_start(den0[:], den[64:65, :])
                    den_bc = wp.tile([64, 256], F32, tag="den_bc")
                    nc.gpsimd.partition_broadcast(den_bc[:], den0[:],
                                                  channels=64)
                    if debug and p == 0 and h == 0:
                        nc.sync.dma_start(dbg["dbg_den"], den[:])
                        nc.sync.dma_start(dbg["dbg_denbc"], den_bc[:])
                    if h % 2 == 0:
                        nc.vector.tensor_tensor(out=attn[0:64, h // 2, :],
                                                in0=aps[0:64, :], in1=den_bc[:],
                                                op=MUL)
                    else:
                        odd = wp.tile([64, 256], BF16, tag="odd")
                        nc.vector.tensor_tensor(out=odd[:], in0=aps[0:64, :],
                                                in1=den_bc[:], op=MUL)
                        nc.sync.dma_start(attn[64:128, h // 2, :], odd[:])

            def dump_attn(p):
                if debug and p == 0:
                    a = attn_pairs[0]
                    dbg_a = wp.tile([128, 4, 256], F32, tag="dbg_a")
                    nc.vector.tensor_copy(dbg_a[:], a[:])
                    nc.sync.dma_start(dbg["dbg_attn"], dbg_a[:])

            def out_proj(tt):
                attn = attn_pairs[tt // 2]
                half = (tt % 2) * 128
                for c in range(OUT_CH):
                    ops = ps_op.tile([128, OCH], F32, tag="ops")
                    for kt in range(4):
                        nc.tensor.matmul(ops[:], attn[:, kt, half:half + 128],
                                         wo_sb[:, kt, c * OCH:(c + 1) * OCH],
                                         start=(kt == 0), stop=(kt == 3))
                    o_sb = wp.tile([128, OCH], F32, tag="o_sb")
                    nc.vector.tensor_copy(o_sb[:], ops[:])
                    nc.sync.dma_start(
                        po_d[tt * 128:(tt + 1) * 128, c * OCH:(c + 1) * OCH],
                        o_sb[:])

            def whole_body():
                for tt in range(N_TT):
                    produce_tile(tt)
                    if tt % 2 == 1:
                        attention_pair(tt // 2)
                        dump_attn(tt // 2)
                        out_proj(tt - 1)
                        out_proj(tt)

            if loop_n == 1:
                whole_body()
            else:
                def loop_body(_iv):
                    for i in range(N_TT):
                        kT_tiles[i] = None
                        vA_tiles[i] = None
                    for i in range(N_PAIR):
                        qT_pairs[i] = None
                        attn_pairs[i] = None
                    whole_body()
                with tc.For_i(0, loop_n, 1) as iv:
                    loop_body(iv)

    nc.compile()
    return nc


def _host_inputs(x, norm_scale, qkv_w, qkv_b, out_w, out_b, sinks):
    assert np.allclose(np.asarray(qkv_b), 0.0), "nonzero qkv_b unsupported"
    x = np.asarray(x, dtype=np.float32)
    norm_scale = np.asarray(norm_scale, dtype=np.float32)
    qkv_w = np.asarray(qkv_w, dtype=np.float32)
    out_w = np.asarray(out_w, dtype=np.float32)
    sinks = np.asarray(sinks, dtype=np.float32)

    import ml_dtypes
    x_pad = np.zeros((N_TOKENS, HID_PAD), np.float32)
    x_pad[:, :HIDDEN] = x
    wq_fold = norm_scale[:, None] * qkv_w  # fold rmsnorm scale
    cos, sin = _rope_tables()
    mask3 = _mask3()
    ident = np.eye(128, dtype=np.float32)
    cos_q = cos * np.float32(SM_SCALE)
    sin_q = sin * np.float32(SM_SCALE)

    in_maps = []
    for c in range(N_CORES):
        wq_c = np.zeros((HID_PAD, W_G_COLS), np.float32)
        wq_c[:HIDDEN, 0:GRP] = wq_fold[:, c * GRP:(c + 1) * GRP]
        wq_c[:HIDDEN, GRP:GRP + HEAD_DIM] = \
            wq_fold[:, Q_COLS + c * HEAD_DIM:Q_COLS + (c + 1) * HEAD_DIM]
        wq_c[:HIDDEN, GRP + HEAD_DIM:] = \
            wq_fold[:, Q_COLS + KV_COLS + c * HEAD_DIM:
                    Q_COLS + KV_COLS + (c + 1) * HEAD_DIM]
        wo_c = out_w[c * GRP:(c + 1) * GRP, :].astype(ml_dtypes.bfloat16)
        es_c = np.broadcast_to(
            np.exp(sinks[c * Q_MULT:(c + 1) * Q_MULT])[None, :],
            (128, Q_MULT)).copy().astype(np.float32)
        in_maps.append({
            "x": x_pad, "w_qkv": wq_c, "w_out": wo_c,
            "cos_q": cos_q, "sin_q": sin_q, "cos_k": cos, "sin_k": sin,
            "mask3": mask3, "esink": es_c, "ident": ident,
            "ones": np.ones((128, 1), np.float32),
        })
    return in_maps


def kernel(x, norm_scale, qkv_w, qkv_b, out_w, out_b, sinks):
    from concourse import bass_utils
    if "nc" not in _CACHE:
        _CACHE["nc"] = _build_program()
    nc = _CACHE["nc"]
    in_maps = _host_inputs(x, norm_scale, qkv_w, qkv_b, out_w, out_b, sinks)
    res = bass_utils.run_bass_kernel_spmd(nc, in_maps, core_ids=list(range(N_CORES)))
    acc = np.asarray(x, dtype=np.float32).copy()
    for c in range(N_CORES):
        acc += res.results[c]["pout"]
    acc += np.asarray(out_b, dtype=np.float32)[None, :]
    return acc
